# revision 44
# baseline (speedup 1.0000x reference)
"""Colorizer kernel for Trainium2 (8 NeuronCores, SPMD).

out[b,c,y,x] = sum_p softmax_p(corr[b,p,y,x]) * one_hot(labels)[c, y+dy, x+dx]
over a 13x13 displacement window; corr = <feats_t[:,y,x], feats_r[:,y+dy,x+dx]>
over 256 channels; out-of-bounds displacements get zero weight.

Sharding: core = half*4 + batch. Each core: 64 query rows. The bottom half is
y-MIRRORED on host so all 8 cores run one identical SPMD program (the 13x13
window and band mask are y-symmetric).

Design (measured on HW, ~87us vs 259us baseline):
 - fp16 t/r (host-converted; gram accumulates fp32 in PSUM, rel err 6.3e-3
   vs the 2e-2 budget).
 - t loaded as 3 overlapping y-slabs; gram matmuls stream STRIDED 3D
   windows straight from the slab tiles (no staging DMAs, which were the
   baseline bottleneck: 131k x 112B descriptors).
 - input DMAs ride a priority-ordered sync queue (DMA engines round-robin
   across queues, so queue order is the only priority knob); the one-hot
   table rides the otherwise-idle scalar queue in parallel.
 - natural (unpadded) windows; oht ones at cols 0:16 put the softmax
   denominator at PSUM partition 0 (reciprocal_approx_fast ucode breaks at
   nonzero base partition); classes at cols 32:48.
 - agg accumulates into y-major strip PSUM [48, 8, 128] with 4-row bank
   pieces over the full x-window: strided MM outputs with inner-contiguous
   runs stream at full rate (x-major 4B-run outputs were ~2x slower).
 - strip aggregation is emitted one block late so the PE fills mask-wait
   gaps with the next block's gram; band-mask multiplies are split between
   VectorE and GpSimd; output DMAs launch from the gpsimd queue so they
   never head-of-line block input transfers or the scalar exp stream.
"""
import sys
sys.path.insert(0, "/opt/trn_rl_repo")

import numpy as np
import ml_dtypes

D, R, C = 4, 6, 16
B, CF, H1, W1 = 4, 256, 128, 128
HALF = 64
NBR = 9             # key block-rows per core (72 key rows)
NXB = 8             # x-blocks per row (16 key cols each)
BIAS = -64.0
EPAD = 576
TSPANS = [(0, 22, (0, 1)), (10, 46, (2, 3, 4)), (34, 64, (5, 6, 7, 8))]
GP_XB = {0, 7}  # band-mask multiplies routed to GpSimd

_COMPILED = None


def _windows():
    out = []
    for k in range(NBR):
        ky0 = 8 * k
        a0 = max(0, ky0 - 6)
        b0 = min(HALF, ky0 + 14)
        rows = b0 - a0
        row = []
        for xb in range(NXB):
            kx0 = 16 * xb
            xlo = max(0, kx0 - 6)
            xhi = min(W1, kx0 + 22)
            nx = xhi - xlo
            if rows * nx <= 512:
                pieces = [(a0, rows)]
            else:
                h = rows // 2
                assert h * nx <= 512, (k, xb, rows, nx)
                pieces = [(a0, h), (a0 + h, rows - h)]
            m_a = a0 - ky0 + 12
            xr = xlo - (16 * xb - 6)
            row.append(dict(ky0=ky0, a0=a0, b0=b0, rows=rows,
                            xlo=xlo, nx=nx, pieces=pieces, m_a=m_a, xr=xr))
        out.append(row)
    return out


def _mask_combos(win):
    """Distinct (m_a, rows, xr, nx) band-mask shapes -> contiguous offsets."""
    offs = {}
    off = 0
    for k in range(NBR):
        for xb in range(NXB):
            w = win[k][xb]
            key = (w['m_a'], w['rows'], w['xr'], w['nx'])
            if key not in offs:
                offs[key] = off
                off += w['rows'] * w['nx']
    return offs, off


def _build():
    import concourse.tile as tile
    import concourse.mybir as mybir
    from concourse import bacc
    from contextlib import ExitStack

    f32 = mybir.dt.float32
    f16 = mybir.dt.float16
    bf16 = mybir.dt.bfloat16
    Exp = mybir.ActivationFunctionType.Exp

    win = _windows()
    moffs, mtot = _mask_combos(win)

    nc = bacc.Bacc("TRN2", target_bir_lowering=False, debug=False, num_devices=8)
    t_d = nc.dram_tensor("t", [CF, HALF * W1], f16, kind="ExternalInput").ap()
    r_d = nc.dram_tensor("r", [CF, NBR * 8 * W1], f16, kind="ExternalInput").ap()
    oht_d = nc.dram_tensor("oht", [128, NBR * NXB * 48], bf16,
                           kind="ExternalInput").ap()
    msk_d = nc.dram_tensor("msk", [128, mtot], bf16, kind="ExternalInput").ap()
    out_d = nc.dram_tensor("out", [C, HALF, W1], f32, kind="ExternalOutput").ap()

    with tile.TileContext(nc) as tc, ExitStack() as ctx:
        const_p = ctx.enter_context(tc.tile_pool(name="const", bufs=1))
        t_p = ctx.enter_context(tc.tile_pool(name="tbuf", bufs=1))
        r_p = ctx.enter_context(tc.tile_pool(name="rbuf", bufs=4))
        e_p = ctx.enter_context(tc.tile_pool(name="ebuf", bufs=6))
        st_p = ctx.enter_context(tc.tile_pool(name="stage", bufs=3))
        gps = ctx.enter_context(tc.tile_pool(name="gram", bufs=2, space="PSUM"))
        aps = ctx.enter_context(tc.tile_pool(name="aggp", bufs=2, space="PSUM"))

        # all input loads on the SYNC queue in strict priority order (the
        # DMA engines round-robin across queues, so a single ordered queue
        # is the only way to make the first-needed bytes arrive first).
        t_tiles = {}

        def load_slab_ch(si, ch):
            ra, rb, _ = TSPANS[si]
            tl = t_p.tile([128, (rb - ra) * W1], f16, name=f"t{si}_{ch}")
            nc.sync.dma_start(
                tl[:], t_d[ch * 128:(ch + 1) * 128, ra * W1:rb * W1])
            t_tiles[(si, ch)] = tl[:].rearrange("p (y x) -> p y x",
                                                y=rb - ra)

        r_tiles = {}

        def load_r_ch(k, ch):
            if k not in r_tiles:
                r_tiles[k] = [r_p.tile([128, 8 * W1], f16, tag=f"r{ch2}",
                                       name=f"r{ch2}_{k}") for ch2 in (0, 1)]
            nc.sync.dma_start(
                r_tiles[k][ch][:],
                r_d[ch * 128:(ch + 1) * 128, k * 8 * W1:(k + 1) * 8 * W1])

        load_slab_ch(0, 0)
        load_r_ch(0, 0)
        load_r_ch(0, 1)
        load_slab_ch(0, 1)
        load_r_ch(1, 0)
        load_r_ch(1, 1)
        msk_t = const_p.tile([128, mtot], bf16)
        nc.sync.dma_start(msk_t[:], msk_d[:])
        oht_t = const_p.tile([128, NBR * NXB * 48], bf16)
        nc.scalar.dma_start(oht_t[:], oht_d[:])
        load_slab_ch(1, 0)
        load_slab_ch(1, 1)
        load_r_ch(2, 0)
        load_r_ch(2, 1)
        load_r_ch(3, 0)
        load_r_ch(3, 1)
        load_slab_ch(2, 0)
        load_slab_ch(2, 1)
        load_r_ch(4, 0)
        load_r_ch(4, 1)
        span_of = {}
        for si, (ra, rb, ks) in enumerate(TSPANS):
            for k in ks:
                span_of[k] = (si, ra)

        bias_t = const_p.tile([128, 1], f32)
        nc.vector.memset(bias_t[:], BIAS)

        e_tiles = {}
        strip_after = {}
        for s in range(HALF // 8):
            ks = [k for k in range(NBR)
                  if win[k][0]['a0'] < 8 * s + 8 and win[k][0]['b0'] > 8 * s]
            strip_after[s] = max(ks)

        def do_strip(s):
            # y-major strip PSUM [48, 8 rows, 128 x]: inner-x-contiguous
            # strided MM outputs stream at full rate (x-major 4B-run outs
            # were ~2x slower); bank boundary every 4 rows.
            pt = aps.tile([48, 1024], f32, tag="aggps")
            pt3 = pt[:].rearrange("p (r x) -> p r x", r=8)
            started = [False, False]
            for k in range(NBR):
                w0 = win[k][0]
                if not (w0['a0'] < 8 * s + 8 and w0['b0'] > 8 * s):
                    continue
                et = e_tiles[k]
                for xb in range(NXB):
                    w = win[k][xb]
                    ra = max(w['a0'], 8 * s)
                    rb = min(w['b0'], 8 * s + 8)
                    if ra >= rb:
                        continue
                    nx, xlo = w['nx'], w['xlo']
                    e3 = et[:, xb * EPAD:xb * EPAD + w['rows'] * nx].rearrange(
                        "p (r x) -> p r x", r=w['rows'])
                    lin = k * NXB + xb
                    for (pa, pb) in ((ra, min(rb, 8 * s + 4)),
                                     (max(ra, 8 * s + 4), rb)):
                        if pa >= pb:
                            continue
                        bank = (pa - 8 * s) // 4
                        rhs = e3[:, pa - w['a0']:pb - w['a0'], :]
                        o = pt3[:, pa - 8 * s:pb - 8 * s, xlo:xlo + nx]
                        nc.tensor.matmul(
                            o, oht_t[:, lin * 48:(lin + 1) * 48], rhs,
                            start=not started[bank], stop=False)
                        started[bank] = True
            # denominator at PSUM partitions 0:16 (oht ones at cols 0:16);
            # reciprocal_approx_fast ucode needs base partition 0.
            rec = st_p.tile([16, 1024], f32, tag="rec")
            nc.vector.reciprocal_approx_fast(rec[:], pt[0:16, :])
            stg = st_p.tile([16, 1024], f32, tag="stg")
            nc.vector.tensor_mul(stg[:], pt[32:48, :], rec[:])
            nc.gpsimd.dma_start(
                out_d[:, 8 * s:8 * s + 8, :],
                stg[:].rearrange("p (r x) -> p r x", r=8))

        for k in range(NBR):
            # host pre-arranged r block-major: [c, k, xb, ky*16+kx], fp16
            if k + 5 <= NBR - 1 and (k + 5) not in r_tiles:
                load_r_ch(k + 5, 0)
                load_r_ch(k + 5, 1)
            r_t = r_tiles[k]
            et = e_p.tile([128, NXB * EPAD], bf16, tag="E")
            e_tiles[k] = et
            si, sra = span_of[k]
            for xb in range(NXB):
                w = win[k][xb]
                rows, nx, xlo = w['rows'], w['nx'], w['xlo']
                ntot = rows * nx
                gp = gps.tile([128, 1024], f32, tag="G")
                for ch in (0, 1):
                    lhsT = r_t[ch][:, 128 * xb:128 * xb + 128]
                    t3 = t_tiles[(si, ch)]
                    for i, (pa, pr) in enumerate(w['pieces']):
                        rhs = t3[:, pa - sra:pa - sra + pr, xlo:xlo + nx]
                        o = gp[:, 512 * i:512 * i + pr * nx]
                        nc.tensor.matmul(o, lhsT, rhs, start=(ch == 0),
                                         stop=(ch == 1))
                eo = et[:, xb * EPAD:xb * EPAD + ntot]
                if len(w['pieces']) == 1:
                    ei = gp[:, 0:ntot]
                else:
                    h = w['pieces'][0][1]
                    ei = gp[:].rearrange("p (t u) -> p t u", t=2)[:, :, 0:h * nx]
                    eo = eo.rearrange("p (t u) -> p t u", t=2)
                nc.scalar.activation(eo, ei, Exp, bias=bias_t[:], scale=1.0)
                mo = moffs[(w['m_a'], rows, w['xr'], nx)]
                ef = et[:, xb * EPAD:xb * EPAD + ntot]
                eng = nc.gpsimd if (xb in GP_XB and k < NBR - 1) else nc.vector
                eng.tensor_mul(ef, ef, msk_t[:, mo:mo + ntot])
            # strips one block late: the PE fills the mask-wait with gram k
            for s in range(HALF // 8):
                if strip_after[s] == k - 1:
                    do_strip(s)
        for s in range(HALF // 8):
            if strip_after[s] == NBR - 1:
                do_strip(s)
    nc.compile()
    return nc


def _prep_host(quantized_r):
    q = quantized_r[:, 0]
    a = q.reshape(B, H1, 4, 512)[:, :, 1:3, :].sum(2)
    s = a.reshape(B, H1, W1, 4)[:, :, :, 1:3].sum(3)
    # CPU-jax reference semantics: f32->i32 convert truncates (values >= 0)
    return s // 4


def _mask_host():
    win = _windows()
    moffs, mtot = _mask_combos(win)
    ky = np.arange(128) // 16
    kx = np.arange(128) % 16
    m = np.zeros((128, mtot), np.float32)
    for (m_a, rows, xr, nx), off in moffs.items():
        mi = m_a + np.arange(rows)
        rx = xr + np.arange(nx)
        blk = ((np.abs(mi[None, :, None] - 12 - ky[:, None, None]) <= 6)
               & (np.abs(rx[None, None, :] - 6 - kx[:, None, None]) <= 6))
        m[:, off:off + rows * nx] = blk.reshape(128, rows * nx)
    return m.astype(ml_dtypes.bfloat16)


def _oht_host(labels_half):
    o = np.zeros((128, NBR * NXB, 48), np.float32)
    for k in range(NBR):
        for xb in range(NXB):
            lab = labels_half[8 * k:8 * k + 8, 16 * xb:16 * xb + 16].reshape(128)
            o[:, k * NXB + xb, 0:16] = 1.0  # denominator ones (partition 0:16)
            o[np.arange(128), k * NXB + xb, 32 + lab] = 1.0  # classes 32:48
    return o.reshape(128, NBR * NXB * 48).astype(ml_dtypes.bfloat16)


def kernel(feats_r, feats_t, quantized_r):
    global _COMPILED
    from concourse.bass_utils import run_bass_kernel_spmd

    feats_r = np.asarray(feats_r, np.float32)
    feats_t = np.asarray(feats_t, np.float32)
    quantized_r = np.asarray(quantized_r, np.int32)

    if _COMPILED is None:
        _COMPILED = _build()

    labels = _prep_host(quantized_r)
    msk = _mask_host()
    in_maps = []
    for core in range(8):
        half, b = core // 4, core % 4
        if half == 0:
            t = feats_t[b, :, 0:HALF, :]
            r = feats_r[b, :, 0:72, :]
            lab = labels[b, 0:72, :]
        else:  # y-mirrored bottom half
            t = feats_t[b, :, ::-1, :][:, 0:HALF, :]
            r = feats_r[b, :, ::-1, :][:, 0:72, :]
            lab = labels[b, ::-1, :][0:72, :]
        r_bm = np.ascontiguousarray(r).reshape(CF, NBR, 8, NXB, 16) \
            .transpose(0, 1, 3, 2, 4).reshape(CF, 72 * W1)
        in_maps.append(dict(
            t=np.ascontiguousarray(t).reshape(CF, HALF * W1).astype(np.float16),
            r=r_bm.astype(np.float16),
            oht=np.ascontiguousarray(_oht_host(lab)),
            msk=msk,
        ))
    res = run_bass_kernel_spmd(_COMPILED, in_maps, core_ids=list(range(8)))
    kernel._last_res = res
    out = np.empty((B, C, H1, W1), np.float32)
    for core in range(8):
        half, b = core // 4, core % 4
        oc = res.results[core]["out"]
        if half == 0:
            out[b, :, 0:HALF, :] = oc
        else:
            out[b, :, HALF:, :] = oc[:, ::-1, :]
    return out
